# revision 8
# baseline (speedup 1.0000x reference)
"""Trainium2 Bass kernel for a GPT-2-style transformer block.

Shapes (hardcoded): x [8, 1024, 768], 12 heads, head dim 64, MLP hidden 3072,
exact (erf) GELU, LayerNorm eps 1e-5, full (non-causal) attention.

Sharding: data-parallel over batch — core i computes batch element i end to
end; weights are replicated. No collectives.

v2 design:
  - All attention GEMMs (q/k/v projections, AV, output proj) run in fp8
    (e4m3) with DoubleRow perf mode: two 128-row contraction slabs per
    matmul, halving accumulation passes. Weights are pre-scaled by powers
    of two into fp8's normal range; the inverse scales fold into the f32
    PSUM epilogues. expS is stored fp8 via exp(S-3) (constant shift cancels
    in softmax). S matmuls stay bf16: their cost is PSUM-write-bound, fp8
    wouldn't help. MLP stays bf16 (fp8 there costs ~2.1% rel err, over
    budget).
  - After QKV, the block runs in four 256-token quarters: AV + proj + LN2 +
    MLP of quarter Q overlap the ScalarE exp stream of quarter Q+1, keeping
    the PE busy during the softmax phase.
  - Residual stream held in bf16 (x cast on load), halving SBUF footprint.
"""

import numpy as np
import ml_dtypes
from contextlib import ExitStack

N_CORES = 8
N = 1024          # tokens per core
C = 768           # embed
HEADS = 12
D = 64            # head dim
HID = 3072        # mlp hidden
NT = N // 128     # 8 token tiles
FC = C // 128     # 6 feature tiles
KCP = FC // 2     # 3 feature-tile pairs (DoubleRow slabs)
FH = HID // 128   # 24 hidden tiles
NQ = 4            # query/token quarters
QT = N // NQ      # 256 tokens per quarter
EPS = 1e-5
EXP_SHIFT = 3.0   # exp(S - shift); cancels in softmax, keeps expS < fp8 max

# power-of-2 operand scales (fold back in epilogues)
SX = 32.0         # xn going into q/k/v
SWQ = 512.0       # Wq (includes the 1/8 attn scale fold, so tiny values)
SWK = 64.0
SWV = 64.0
SV = 32.0         # v values in vaug (carries through to o)
SWO = 64.0
SQK = 32.0        # q/k rows as stored (fp8); S logits come out x1024

_CACHE = {}


def _build():
    import concourse.bass as bass
    import concourse.tile as tile
    from concourse import bacc, mybir
    from concourse.masks import make_identity

    f32 = mybir.dt.float32
    bf16 = mybir.dt.bfloat16
    f8 = mybir.dt.float8e4
    AF = mybir.ActivationFunctionType
    ALU = mybir.AluOpType
    PM = mybir.MatmulPerfMode

    nc = bacc.Bacc("TRN2", target_bir_lowering=False, debug=False,
                   num_devices=N_CORES)

    x_d = nc.dram_tensor("x", [N, C], f32, kind="ExternalInput").ap()
    wq_d = nc.dram_tensor("wq", [C, C], f8, kind="ExternalInput").ap()
    wk_d = nc.dram_tensor("wk", [C, C], f8, kind="ExternalInput").ap()
    wv_d = nc.dram_tensor("wv", [C, C], f8, kind="ExternalInput").ap()
    wo_d = nc.dram_tensor("wo", [C, C], f8, kind="ExternalInput").ap()
    w1_d = nc.dram_tensor("w1", [C, HID], bf16, kind="ExternalInput").ap()
    w2_d = nc.dram_tensor("w2", [HID, C], bf16, kind="ExternalInput").ap()
    bq_d = nc.dram_tensor("bq", [C], f32, kind="ExternalInput").ap()
    bk_d = nc.dram_tensor("bk", [C], f32, kind="ExternalInput").ap()
    b1_d = nc.dram_tensor("b1", [HID], f32, kind="ExternalInput").ap()
    bv32_d = nc.dram_tensor("bv32", [C], bf16, kind="ExternalInput").ap()
    bo_d = nc.dram_tensor("bo", [C], bf16, kind="ExternalInput").ap()
    b2_d = nc.dram_tensor("b2", [C], bf16, kind="ExternalInput").ap()
    ind2_d = nc.dram_tensor("ind2", [2, 128], bf16, kind="ExternalInput").ap()
    out_d = nc.dram_tensor("out", [N, C], f32, kind="ExternalOutput").ap()

    with tile.TileContext(nc) as tc, ExitStack() as ctx:
        # ---------------- persistent pools ----------------
        consts = ctx.enter_context(tc.tile_pool(name="consts", bufs=1))
        xpool = ctx.enter_context(tc.tile_pool(name="xres", bufs=NT))
        stat_pool = ctx.enter_context(tc.tile_pool(name="stats", bufs=4))

        ident = consts.tile([128, 128], bf16, tag="ident")
        make_identity(nc, ident)
        ind2 = consts.tile([2, 128], bf16, tag="ind2")
        nc.sync.dma_start(ind2[:], ind2_d[:])

        eps_t = consts.tile([128, 1], f32, tag="eps")
        nc.vector.memset(eps_t[:], EPS)
        shift_t = consts.tile([128, 1], f32, tag="shift")
        nc.vector.memset(shift_t[:], -EXP_SHIFT)
        warm_t = consts.tile([128, 1], f32, tag="warm")
        nc.scalar.activation(warm_t[:], eps_t[:], AF.Sqrt)  # preload table

        # per-partition bias columns for feature-major evictions
        bqc = consts.tile([128, FC], f32, tag="bqc")
        nc.sync.dma_start(bqc[:], bq_d.rearrange("(m p) -> p m", p=128))
        bkc = consts.tile([128, FC], f32, tag="bkc")
        nc.sync.dma_start(bkc[:], bk_d.rearrange("(m p) -> p m", p=128))
        b1c = consts.tile([128, FH], f32, tag="b1c")
        nc.sync.dma_start(b1c[:], b1_d.rearrange("(m p) -> p m", p=128))

        # partition-broadcast bias rows (bf16) for token-major additions
        bv32_b = consts.tile([128, C], bf16, tag="bv32_b")
        nc.sync.dma_start(bv32_b[:], bv32_d.partition_broadcast(128))
        bo_b = consts.tile([128, C], bf16, tag="bo_b")
        nc.sync.dma_start(bo_b[:], bo_d.partition_broadcast(128))
        b2_b = consts.tile([128, C], bf16, tag="b2_b")
        nc.sync.dma_start(b2_b[:], b2_d.partition_broadcast(128))

        # residual-carrying x tiles (bf16), live whole kernel
        xt = [xpool.tile([128, C], bf16, tag="xt", name="xt") for _ in range(NT)]

        # ================= head phase: load x, LN1, QKV =================
        h_stack = ExitStack()
        xf_pool = h_stack.enter_context(tc.tile_pool(name="xf32", bufs=2))
        xnT_pool = h_stack.enter_context(tc.tile_pool(name="xnT8", bufs=KCP))
        wqk_pool = h_stack.enter_context(tc.tile_pool(name="wqk", bufs=2 * KCP))
        wv_pool = h_stack.enter_context(tc.tile_pool(name="wv8", bufs=KCP))
        psH = h_stack.enter_context(
            tc.tile_pool(name="psH", bufs=4, space="PSUM"))
        tmpH = h_stack.enter_context(tc.tile_pool(name="tmpH", bufs=2))

        def load_x(mt):
            xf = xf_pool.tile([128, C], f32, tag="xf", name="xf")
            nc.sync.dma_start(xf[:], x_d[mt * 128:(mt + 1) * 128, :])
            nc.vector.tensor_copy(xt[mt][:], xf[:])

        for mt in range(4):
            load_x(mt)

        wq_sb = [wqk_pool.tile([128, 2, C], f8, tag="wqk", name="wqk")
                 for _ in range(KCP)]
        wk_sb = [wqk_pool.tile([128, 2, C], f8, tag="wqk", name="wqk")
                 for _ in range(KCP)]
        wv_sb = [wv_pool.tile([128, 2, C], f8, tag="wv8", name="wv8")
                 for _ in range(KCP)]
        for kcp in range(KCP):
            nc.sync.dma_start(
                wq_sb[kcp][:],
                wq_d[kcp * 256:(kcp + 1) * 256, :].rearrange(
                    "(two p) m -> p two m", two=2))
            nc.sync.dma_start(
                wk_sb[kcp][:],
                wk_d[kcp * 256:(kcp + 1) * 256, :].rearrange(
                    "(two p) m -> p two m", two=2))
        for mt in range(4, NT):
            load_x(mt)
        for kcp in range(KCP):
            nc.sync.dma_start(
                wv_sb[kcp][:],
                wv_d[kcp * 256:(kcp + 1) * 256, :].rearrange(
                    "(two p) m -> p two m", two=2))

        # persistent attention operands (right SBUF stack: outlive head pools)
        a_stack = ExitStack()
        qkT_pool = a_stack.enter_context(
            tc.tile_pool(name="qkT", bufs=2 * FC, side="right"))
        vaug_pool = a_stack.enter_context(
            tc.tile_pool(name="vaug8", bufs=1, side="right"))
        qT = [qkT_pool.tile([128, N], f8, tag="qT", name="qT")
              for _ in range(FC)]
        kT = [qkT_pool.tile([128, N], f8, tag="kT", name="kT")
              for _ in range(FC)]
        # head pitch padded 65->68: DoubleRow weight slab stride must be 16B-aligned
        vaug8 = vaug_pool.tile([128, NT, HEADS, 68], f8, tag="vaug8")

        xnT8 = [xnT_pool.tile([128, 2, N], f8, tag="xnT8", name="xnT8")
                for _ in range(KCP)]

        def ln_tiles(src_list, mt0, dstT, rstd_scale, tmp_pool, ps_pool,
                     ps_tag):
            """LayerNorm + PE-transpose to feature-major.

            dstT(fc) -> (tile, col-slice resolver) for the eviction target."""
            for i, src in enumerate(src_list):
                mt = mt0 + i
                st = stat_pool.tile([128, 3, 6], f32, tag="bnst")
                sub = src[:].rearrange("p (s d) -> p s d", s=3)
                for s in range(3):
                    nc.vector.bn_stats(st[:, s, :], sub[:, s, :])
                mv = stat_pool.tile([128, 2], f32, tag="bnmv")
                nc.vector.bn_aggr(mv[:], st[:])
                sd = stat_pool.tile([128, 1], f32, tag="bnsd")
                nc.scalar.activation(sd[:], mv[:, 1:2], AF.Sqrt, bias=eps_t[:])
                rstd = stat_pool.tile([128, 1], f32, tag="bnrs")
                nc.vector.reciprocal(rstd[:], sd[:])
                if rstd_scale != 1.0:
                    rs2 = stat_pool.tile([128, 1], f32, tag="bnr2")
                    nc.vector.tensor_scalar_mul(rs2[:], rstd[:], rstd_scale)
                    rstd = rs2
                xn = tmp_pool.tile([128, C], bf16, tag="xn", name="xn")
                nc.vector.tensor_scalar(
                    out=xn[:], in0=src[:],
                    scalar1=mv[:, 0:1], scalar2=rstd[:],
                    op0=ALU.subtract, op1=ALU.mult)
                for fc in range(FC):
                    pt = ps_pool.tile([128, 128], bf16, tag=ps_tag,
                                      name="psT")
                    nc.tensor.transpose(pt[:], xn[:, fc * 128:(fc + 1) * 128],
                                        ident[:])
                    dtile, csl = dstT(fc)
                    nc.vector.tensor_copy(
                        dtile[csl(mt * 128, (mt + 1) * 128)], pt[:])

        def ln1_dst(fc):
            return (xnT8[fc // 2],
                    lambda a, b, i=fc % 2: np.s_[:, i, a:b])

        def qk_block(nb):
            for w_sb, bias_col, dstT, sc in (
                    (wq_sb, bqc, qT, SQK / (SX * SWQ)),
                    (wk_sb, bkc, kT, SQK / (SX * SWK))):
                for mc in range(FC):
                    ps = psH.tile([128, 512], f32, tag="psH", name="psqk")
                    for kcp in range(KCP):
                        nc.tensor.matmul(
                            ps[:],
                            w_sb[kcp][:, :, mc * 128:(mc + 1) * 128],
                            xnT8[kcp][:, :, nb * 512:(nb + 1) * 512],
                            start=(kcp == 0), stop=(kcp == KCP - 1),
                            perf_mode=PM.DoubleRow)
                    nc.vector.tensor_scalar(
                        out=dstT[mc][:, nb * 512:(nb + 1) * 512], in0=ps[:],
                        scalar1=sc, scalar2=bias_col[:, mc:mc + 1],
                        op0=ALU.mult, op1=ALU.add)

        def v_block(mt):
            for nb in range(2):          # 6 heads (384 cols) per block
                ps = psH.tile([128, 384], f32, tag="psH", name="psv")
                for kcp in range(KCP):
                    nc.tensor.matmul(
                        ps[:],
                        xnT8[kcp][:, :, mt * 128:(mt + 1) * 128],
                        wv_sb[kcp][:, :, nb * 384:(nb + 1) * 384],
                        start=(kcp == 0), stop=(kcp == KCP - 1),
                        perf_mode=PM.DoubleRow)
                nc.vector.scalar_tensor_tensor(
                    vaug8[:, mt, nb * 6:(nb + 1) * 6, 0:D],
                    ps[:].rearrange("p (h e) -> p h e", h=6),
                    SV / (SX * SWV),
                    bv32_b[:, nb * 384:(nb + 1) * 384].rearrange(
                        "p (h e) -> p h e", h=6),
                    ALU.mult, ALU.add)
            nc.vector.memset(vaug8[:, mt, :, D:D + 1], 1.0)

        ln_tiles(xt[0:4], 0, ln1_dst, SX, tmpH, psH, "psHT")
        qk_block(0)
        ln_tiles(xt[4:8], 4, ln1_dst, SX, tmpH, psH, "psHT")
        qk_block(1)
        for mt in range(NT):
            v_block(mt)
        h_stack.close()   # frees xf32, xnT8, wq/wk/wv, psH, tmpH

        # proj + MLP weights (DMAs run during attention)
        w_stack = ExitStack()
        wo_pool = w_stack.enter_context(tc.tile_pool(name="wo8", bufs=KCP))
        w1_pool = w_stack.enter_context(tc.tile_pool(name="w1", bufs=FC))
        w2_pool = w_stack.enter_context(tc.tile_pool(name="w2", bufs=FH))
        wo_sb = [wo_pool.tile([128, 2, C], f8, tag="wo8", name="wo8")
                 for _ in range(KCP)]
        for kcp in range(KCP):
            nc.sync.dma_start(
                wo_sb[kcp][:],
                wo_d[kcp * 256:(kcp + 1) * 256, :].rearrange(
                    "(two p) m -> p two m", two=2))
        w1_sb = [w1_pool.tile([128, HID], bf16, tag="w1", name="w1")
                 for _ in range(FC)]
        for kc in range(FC):
            nc.sync.dma_start(w1_sb[kc][:], w1_d[kc * 128:(kc + 1) * 128, :])
        w2_sb = [w2_pool.tile([128, C], bf16, tag="w2", name="w2")
                 for _ in range(FH)]
        for kc in range(FH):
            nc.sync.dma_start(w2_sb[kc][:], w2_d[kc * 128:(kc + 1) * 128, :])

        # ================= main loop pools =================
        # PSUM bank budget (8): psS 3 + psA 2 + psX 3
        m_stack = ExitStack()
        e_pool = m_stack.enter_context(tc.tile_pool(name="expS8", bufs=13))
        oa_pool = m_stack.enter_context(tc.tile_pool(name="oa", bufs=6))
        on_pool = m_stack.enter_context(tc.tile_pool(name="oTn8", bufs=2 * KCP))
        rr_pool = m_stack.enter_context(tc.tile_pool(name="rrec", bufs=2))
        xn2_pool = m_stack.enter_context(tc.tile_pool(name="xn2T", bufs=FC))
        h8_pool = m_stack.enter_context(tc.tile_pool(name="hT", bufs=FH))
        ot_pool = m_stack.enter_context(tc.tile_pool(name="outs", bufs=2))
        tmpM = m_stack.enter_context(tc.tile_pool(name="tmpM", bufs=2))
        psS = m_stack.enter_context(
            tc.tile_pool(name="psS", bufs=4, space="PSUM"))
        psA = m_stack.enter_context(
            tc.tile_pool(name="psA", bufs=2, space="PSUM"))
        psX = m_stack.enter_context(
            tc.tile_pool(name="psX", bufs=2, space="PSUM"))

        def s_exp(h, q, expS):
            """S^T then exp for head h, query quarter q -> expS [128,NT,QT]."""
            mc, off = h // 2, (h % 2) * D
            for kth in range(4):
                ps = psS.tile([128, 2, QT], f32, tag="psS", name="psS")
                for k2 in range(2):
                    kt = 2 * kth + k2
                    nc.tensor.matmul(
                        ps[:, k2, :],
                        kT[mc][off:off + D, kt * 128:(kt + 1) * 128],
                        qT[mc][off:off + D, q * QT:(q + 1) * QT],
                        start=True, stop=True)
                nc.scalar.activation(
                    expS[:, 2 * kth:2 * kth + 2, :], ps[:], AF.Exp,
                    bias=shift_t[:], scale=1.0 / (SQK * SQK))

        def av(h, expS, oa):
            po = psA.tile([D + 1, QT], f32, tag="psA", name="psO")
            for ktp in range(4):
                nc.tensor.matmul(
                    po[:],
                    vaug8[:, 2 * ktp:2 * ktp + 2, h, 0:D + 1],
                    expS[:, 2 * ktp:2 * ktp + 2, :],
                    start=(ktp == 0), stop=(ktp == 3),
                    perf_mode=PM.DoubleRow)
            nc.vector.tensor_copy(oa[:], po[:])

        def pair_recip(oa_even, oa_odd):
            rs2_bf = rr_pool.tile([2, QT], bf16, tag="rs2b", name="rs2b")
            nc.sync.dma_start(rs2_bf[0:1, :], oa_even[D:D + 1, :])
            nc.sync.dma_start(rs2_bf[1:2, :], oa_odd[D:D + 1, :])
            rs2 = rr_pool.tile([2, QT], f32, tag="rf32", name="rs2")
            nc.vector.tensor_copy(rs2[:], rs2_bf[:])
            rr2 = rr_pool.tile([2, QT], f32, tag="rf32", name="rr2")
            nc.vector.reciprocal_approx_fast(rr2[:], rs2[:])
            rr2_bf = rr_pool.tile([2, QT], bf16, tag="rr2b", name="rr2b")
            nc.vector.tensor_copy(rr2_bf[:], rr2[:])
            return rr2_bf

        def pair_norm(j, oTn_q, oa_even, oa_odd, rr2_bf):
            pb = psA.tile([128, QT], f32, tag="psA", name="psR")
            nc.tensor.matmul(pb[:], ind2[:], rr2_bf[:], start=True, stop=True)
            kcp, i = j // 2, j % 2
            nc.vector.tensor_mul(oTn_q[kcp][0:D, i, :], oa_even[0:D, :],
                                 pb[0:D, :])
            nc.vector.tensor_mul(oTn_q[kcp][D:2 * D, i, :], oa_odd[0:D, :],
                                 pb[D:2 * D, :])

        expS_t = {}

        def emit_s_exp(h, q):
            expS_t[(h, q)] = e = e_pool.tile([128, NT, QT], f8,
                                             tag="expS", name="expS")
            s_exp(h, q, e)

        for h in range(HEADS):
            emit_s_exp(h, 0)

        for q in range(NQ):
            # ---- attention for quarter q (exps already in flight) ----
            for mt01 in range(2):
                mt = 2 * q + mt01
                nc.gpsimd.tensor_add(xt[mt][:], xt[mt][:], bo_b[:])
            oTn_q = [on_pool.tile([128, 2, QT], f8, tag="oTn8", name="oTn8")
                     for _ in range(KCP)]
            oa_t = {}
            rr_prev = None
            for j in range(6):
                for i in range(2):
                    h = 2 * j + i
                    oa_t[h] = oa_pool.tile([D + 1, QT], bf16, tag="oa",
                                           name="oa")
                    av(h, expS_t.pop((h, q)), oa_t[h])
                if j >= 1:
                    pair_norm(j - 1, oTn_q, oa_t.pop(2 * j - 2),
                              oa_t.pop(2 * j - 1), rr_prev)
                rr_prev = pair_recip(oa_t[2 * j], oa_t[2 * j + 1])
                if q + 1 < NQ:
                    emit_s_exp(j, q + 1)   # heads 0-5 of next quarter
            pair_norm(5, oTn_q, oa_t.pop(10), oa_t.pop(11), rr_prev)

            # ---- proj(q) ----
            for mt01 in range(2):
                mt = 2 * q + mt01
                for nb in range(2):
                    ps = psX.tile([128, 384], f32, tag="psX", name="psP")
                    for kcp in range(KCP):
                        nc.tensor.matmul(
                            ps[:],
                            oTn_q[kcp][:, :, mt01 * 128:(mt01 + 1) * 128],
                            wo_sb[kcp][:, :, nb * 384:(nb + 1) * 384],
                            start=(kcp == 0), stop=(kcp == KCP - 1),
                            perf_mode=PM.DoubleRow)
                    nc.vector.scalar_tensor_tensor(
                        xt[mt][:, nb * 384:(nb + 1) * 384], ps[:],
                        1.0 / (SV * SWO),
                        xt[mt][:, nb * 384:(nb + 1) * 384],
                        ALU.mult, ALU.add)

            # ---- LN2(q) -> xn2T ----
            xn2T = [xn2_pool.tile([128, QT], bf16, tag="xn2T", name="xn2T")
                    for _ in range(FC)]

            def ln2_dst(fc):
                return (xn2T[fc],
                        lambda a, b, q=q: np.s_[:, a - 2 * q * 128:
                                                b - 2 * q * 128])

            ln_tiles(xt[2 * q:2 * q + 2], 2 * q, ln2_dst, 1.0, tmpM, psX,
                     "psX")

            # ---- fc1(q) interleaved with S/exp(q+1) for heads 6-11 ----
            hT = [h8_pool.tile([128, QT], bf16, tag="hT", name="hT")
                  for _ in range(FH)]

            def fc1_mc(mc):
                ps = psX.tile([128, QT], f32, tag="psX", name="psF")
                for kc in range(FC):
                    nc.tensor.matmul(
                        ps[:],
                        w1_sb[kc][:, mc * 128:(mc + 1) * 128],
                        xn2T[kc][:],
                        start=(kc == 0), stop=(kc == FC - 1))
                nc.scalar.activation(
                    hT[mc][:], ps[:], AF.Gelu, bias=b1c[:, mc:mc + 1])

            for mc in range(FH):
                fc1_mc(mc)

            # ---- fc2(q) + residual + out, interleaved with S(h6-11, q+1)
            # (gelus precede these exps on ScalarE: one table switch each) ----
            def fc2_group(mt01, nb):
                mt = 2 * q + mt01
                ps = psX.tile([128, 384], f32, tag="psX", name="psF2")
                for kc in range(FH):
                    nc.tensor.matmul(
                        ps[:],
                        hT[kc][:, mt01 * 128:(mt01 + 1) * 128],
                        w2_sb[kc][:, nb * 384:(nb + 1) * 384],
                        start=(kc == 0), stop=(kc == FH - 1))
                ot = ot_pool.tile([128, 384], f32, tag="outs", name="outs")
                nc.vector.tensor_add(
                    ot[:], ps[:], xt[mt][:, nb * 384:(nb + 1) * 384])
                nc.sync.dma_start(
                    out_d[mt * 128:(mt + 1) * 128,
                          nb * 384:(nb + 1) * 384], ot[:])

            fc2_seq = [(0, 0), (0, 1), (1, 0), (1, 1)]
            for step in range(6):
                if q + 1 < NQ:
                    emit_s_exp(6 + step, q + 1)   # heads 6-11 of next quarter
                if step < 4:
                    fc2_group(*fc2_seq[step])

        m_stack.close()
        w_stack.close()
        a_stack.close()

    nc.compile()
    return nc


def _prep_inputs(inputs):
    """Host-side algebraic folds + fp8/bf16 casts. Returns per-core in_maps."""
    f = {k: np.asarray(v, np.float32) for k, v in inputs.items()}
    bf = ml_dtypes.bfloat16
    f8 = ml_dtypes.float8_e4m3
    d = 1.0 / np.sqrt(C // HEADS)

    wq8 = (f["ln1_g"][:, None] * f["Wq"] * d * SWQ).astype(f8)
    bq = ((f["bq"] + f["ln1_b"] @ f["Wq"]) * d * SQK).astype(np.float32)
    wk8 = (f["ln1_g"][:, None] * f["Wk"] * SWK).astype(f8)
    bk = ((f["bk"] + f["ln1_b"] @ f["Wk"]) * SQK).astype(np.float32)
    wv8 = (f["ln1_g"][:, None] * f["Wv"] * SWV).astype(f8)
    bv32 = ((f["bv"] + f["ln1_b"] @ f["Wv"]) * SV).astype(bf)
    wo8 = (f["Wo"] * SWO).astype(f8)
    w1 = (f["ln2_g"][:, None] * f["W1"]).astype(bf)
    b1 = (f["b1"] + f["ln2_b"] @ f["W1"]).astype(np.float32)
    shared = {
        "wq": wq8, "bq": bq, "wk": wk8, "bk": bk, "wv": wv8, "bv32": bv32,
        "wo": wo8, "bo": f["bo"].astype(bf),
        "w1": w1, "b1": b1,
        "w2": f["W2"].astype(bf), "b2": f["b2"].astype(bf),
    }
    ind2 = np.zeros((2, 128), bf)
    ind2[0, 0:64] = 1.0
    ind2[1, 64:128] = 1.0
    shared["ind2"] = ind2
    x = f["x"]
    return [dict(shared, x=np.ascontiguousarray(x[i])) for i in range(N_CORES)]


def kernel(**inputs):
    from concourse.bass_utils import run_bass_kernel_spmd
    if "nc" not in _CACHE:
        _CACHE["nc"] = _build()
    nc = _CACHE["nc"]
    in_maps = _prep_inputs(inputs)
    res = run_bass_kernel_spmd(nc, in_maps, core_ids=list(range(N_CORES)))
    out = np.stack([np.asarray(res.results[i]["out"], np.float32)
                    for i in range(N_CORES)])
    return out


# revision 14
# speedup vs baseline: 1.1382x; 1.1382x over previous
"""Trainium2 Bass kernel for a GPT-2-style transformer block.

Shapes (hardcoded): x [8, 1024, 768], 12 heads, head dim 64, MLP hidden 3072,
exact (erf) GELU, LayerNorm eps 1e-5, full (non-causal) attention.

Sharding: data-parallel over batch — core i computes batch element i end to
end; weights are replicated. No collectives.

v2 design:
  - All attention GEMMs (q/k/v projections, AV, output proj) run in fp8
    (e4m3) with DoubleRow perf mode: two 128-row contraction slabs per
    matmul, halving accumulation passes. Weights are pre-scaled by powers
    of two into fp8's normal range; the inverse scales fold into the f32
    PSUM epilogues. expS is stored fp8 via exp(S-3) (constant shift cancels
    in softmax). S matmuls stay bf16: their cost is PSUM-write-bound, fp8
    wouldn't help. MLP stays bf16 (fp8 there costs ~2.1% rel err, over
    budget).
  - After QKV, the block runs in four 256-token quarters: AV + proj + LN2 +
    MLP of quarter Q overlap the ScalarE exp stream of quarter Q+1, keeping
    the PE busy during the softmax phase.
  - Residual stream held in bf16 (x cast on load), halving SBUF footprint.
"""

import numpy as np
import ml_dtypes
from contextlib import ExitStack

N_CORES = 8
N = 1024          # tokens per core
C = 768           # embed
HEADS = 12
D = 64            # head dim
HID = 3072        # mlp hidden
NT = N // 128     # 8 token tiles
FC = C // 128     # 6 feature tiles
KCP = FC // 2     # 3 feature-tile pairs (DoubleRow slabs)
FH = HID // 128   # 24 hidden tiles
NQ = 4            # query/token quarters
QT = N // NQ      # 256 tokens per quarter
EPS = 1e-5
EXP_SHIFT = 3.0   # exp(S - shift); cancels in softmax, keeps expS < fp8 max

# power-of-2 operand scales (fold back in epilogues)
SX = 32.0         # xn going into q/k/v
SWQ = 512.0       # Wq (includes the 1/8 attn scale fold, so tiny values)
SWK = 64.0
SWV = 64.0
SV = 32.0         # v values in vaug (carries through to o)
SWO = 64.0
SQK = 32.0        # q/k rows as stored (fp8); S logits come out x1024

_CACHE = {}


def _build():
    import concourse.bass as bass
    import concourse.tile as tile
    from concourse import bacc, mybir
    from concourse.masks import make_identity

    f32 = mybir.dt.float32
    bf16 = mybir.dt.bfloat16
    f8 = mybir.dt.float8e4
    AF = mybir.ActivationFunctionType
    ALU = mybir.AluOpType
    PM = mybir.MatmulPerfMode

    nc = bacc.Bacc("TRN2", target_bir_lowering=False, debug=False,
                   num_devices=N_CORES)

    x_d = nc.dram_tensor("x", [N, C], f32, kind="ExternalInput").ap()
    wq_d = nc.dram_tensor("wq", [C, C], f8, kind="ExternalInput").ap()
    wk_d = nc.dram_tensor("wk", [C, C], f8, kind="ExternalInput").ap()
    wv_d = nc.dram_tensor("wv", [C, C], f8, kind="ExternalInput").ap()
    wo_d = nc.dram_tensor("wo", [C, C], f8, kind="ExternalInput").ap()
    w1_d = nc.dram_tensor("w1", [C, HID], bf16, kind="ExternalInput").ap()
    w2_d = nc.dram_tensor("w2", [HID, C], bf16, kind="ExternalInput").ap()
    bq_d = nc.dram_tensor("bq", [C], f32, kind="ExternalInput").ap()
    bk_d = nc.dram_tensor("bk", [C], f32, kind="ExternalInput").ap()
    b1_d = nc.dram_tensor("b1", [HID], f32, kind="ExternalInput").ap()
    bv32_d = nc.dram_tensor("bv32", [C], bf16, kind="ExternalInput").ap()
    bo_d = nc.dram_tensor("bo", [C], bf16, kind="ExternalInput").ap()
    b2_d = nc.dram_tensor("b2", [C], bf16, kind="ExternalInput").ap()
    ind2_d = nc.dram_tensor("ind2", [2, 128], bf16, kind="ExternalInput").ap()
    out_d = nc.dram_tensor("out", [N, C], f32, kind="ExternalOutput").ap()

    with tile.TileContext(nc) as tc, ExitStack() as ctx:
        # ---------------- persistent pools ----------------
        consts = ctx.enter_context(tc.tile_pool(name="consts", bufs=1))
        xpool = ctx.enter_context(tc.tile_pool(name="xres", bufs=NT))
        stat_pool = ctx.enter_context(tc.tile_pool(name="stats", bufs=4))

        ident = consts.tile([128, 128], bf16, tag="ident")
        make_identity(nc, ident)
        ind2 = consts.tile([2, 128], bf16, tag="ind2")
        nc.sync.dma_start(ind2[:], ind2_d[:])

        eps_t = consts.tile([128, 1], f32, tag="eps")
        nc.vector.memset(eps_t[:], EPS)
        shift_t = consts.tile([128, 1], f32, tag="shift")
        nc.vector.memset(shift_t[:], -EXP_SHIFT)
        warm_t = consts.tile([128, 1], f32, tag="warm")
        nc.scalar.activation(warm_t[:], eps_t[:], AF.Sqrt)  # preload table

        # per-partition bias columns for feature-major evictions
        bqc = consts.tile([128, FC], f32, tag="bqc")
        nc.sync.dma_start(bqc[:], bq_d.rearrange("(m p) -> p m", p=128))
        bkc = consts.tile([128, FC], f32, tag="bkc")
        nc.sync.dma_start(bkc[:], bk_d.rearrange("(m p) -> p m", p=128))
        b1c = consts.tile([128, FH], f32, tag="b1c")
        nc.sync.dma_start(b1c[:], b1_d.rearrange("(m p) -> p m", p=128))

        # partition-broadcast bias rows (bf16) for token-major additions
        bv32_b = consts.tile([128, C], bf16, tag="bv32_b")
        nc.sync.dma_start(bv32_b[:], bv32_d.partition_broadcast(128))
        bo_b = consts.tile([128, C], bf16, tag="bo_b")
        nc.sync.dma_start(bo_b[:], bo_d.partition_broadcast(128))
        b2_b = consts.tile([128, C], bf16, tag="b2_b")
        nc.sync.dma_start(b2_b[:], b2_d.partition_broadcast(128))

        # residual-carrying x tiles (bf16), live whole kernel
        xt = [xpool.tile([128, C], bf16, tag="xt", name="xt") for _ in range(NT)]

        # ================= head phase: load x, LN1, QKV =================
        h_stack = ExitStack()
        xf_pool = h_stack.enter_context(tc.tile_pool(name="xf32", bufs=2))
        wqk_pool = h_stack.enter_context(tc.tile_pool(name="wqk", bufs=2 * KCP))
        qk_stack = ExitStack()
        psH = qk_stack.enter_context(
            tc.tile_pool(name="psH", bufs=4, space="PSUM"))
        tmpH = h_stack.enter_context(tc.tile_pool(name="tmpH", bufs=2))

        def load_x(mt):
            xf = xf_pool.tile([128, C], f32, tag="xf", name="xf")
            nc.sync.dma_start(xf[:], x_d[mt * 128:(mt + 1) * 128, :])
            nc.vector.tensor_copy(xt[mt][:], xf[:])

        # persistent attention operands (right SBUF stack: outlive head pools)
        a_stack = ExitStack()
        qkT_pool = a_stack.enter_context(
            tc.tile_pool(name="qkT", bufs=2 * FC, side="right"))
        vaug_pool = a_stack.enter_context(
            tc.tile_pool(name="vaug8", bufs=1, side="right"))
        hv_stack = ExitStack()
        xnT_pool = hv_stack.enter_context(
            tc.tile_pool(name="xnT8", bufs=KCP, side="right"))
        wv_pool = hv_stack.enter_context(
            tc.tile_pool(name="wv8", bufs=KCP, side="right"))
        qT = [qkT_pool.tile([128, N], f8, tag="qT", name="qT")
              for _ in range(FC)]
        kT = [qkT_pool.tile([128, N], f8, tag="kT", name="kT")
              for _ in range(FC)]
        # head pitch padded 65->68: DoubleRow weight slab stride must be 16B-aligned
        vaug8 = vaug_pool.tile([128, NT, HEADS, 68], f8, tag="vaug8")

        xnT8 = [xnT_pool.tile([128, 2, N], f8, tag="xnT8", name="xnT8")
                for _ in range(KCP)]

        for mt in range(4):
            load_x(mt)

        wq_sb = [wqk_pool.tile([128, 2, C], f8, tag="wqk", name="wqk")
                 for _ in range(KCP)]
        wk_sb = [wqk_pool.tile([128, 2, C], f8, tag="wqk", name="wqk")
                 for _ in range(KCP)]
        wv_sb = [wv_pool.tile([128, 2, C], f8, tag="wv8", name="wv8")
                 for _ in range(KCP)]
        for kcp in range(KCP):
            nc.sync.dma_start(
                wq_sb[kcp][:],
                wq_d[kcp * 256:(kcp + 1) * 256, :].rearrange(
                    "(two p) m -> p two m", two=2))
            nc.sync.dma_start(
                wk_sb[kcp][:],
                wk_d[kcp * 256:(kcp + 1) * 256, :].rearrange(
                    "(two p) m -> p two m", two=2))
        for mt in range(4, NT):
            load_x(mt)
        for kcp in range(KCP):
            nc.sync.dma_start(
                wv_sb[kcp][:],
                wv_d[kcp * 256:(kcp + 1) * 256, :].rearrange(
                    "(two p) m -> p two m", two=2))

        def ln_tiles(src_list, mt0, dstT, rstd_scale, tmp_pool, ps_pool,
                     ps_tag):
            """LayerNorm + PE-transpose to feature-major.

            dstT(fc) -> (tile, col-slice resolver) for the eviction target."""
            for i, src in enumerate(src_list):
                mt = mt0 + i
                st = stat_pool.tile([128, 3, 6], f32, tag="bnst")
                sub = src[:].rearrange("p (s d) -> p s d", s=3)
                for s in range(3):
                    nc.vector.bn_stats(st[:, s, :], sub[:, s, :])
                mv = stat_pool.tile([128, 2], f32, tag="bnmv")
                nc.vector.bn_aggr(mv[:], st[:])
                sd = stat_pool.tile([128, 1], f32, tag="bnsd")
                nc.scalar.activation(sd[:], mv[:, 1:2], AF.Sqrt, bias=eps_t[:])
                rstd = stat_pool.tile([128, 1], f32, tag="bnrs")
                nc.vector.reciprocal(rstd[:], sd[:])
                if rstd_scale != 1.0:
                    rs2 = stat_pool.tile([128, 1], f32, tag="bnr2")
                    nc.vector.tensor_scalar_mul(rs2[:], rstd[:], rstd_scale)
                    rstd = rs2
                xn = tmp_pool.tile([128, C], bf16, tag="xn", name="xn")
                nc.vector.tensor_scalar(
                    out=xn[:], in0=src[:],
                    scalar1=mv[:, 0:1], scalar2=rstd[:],
                    op0=ALU.subtract, op1=ALU.mult)
                for fc in range(FC):
                    pt = ps_pool.tile([128, 128], bf16, tag=ps_tag,
                                      name="psT")
                    nc.tensor.transpose(pt[:], xn[:, fc * 128:(fc + 1) * 128],
                                        ident[:])
                    dtile, csl = dstT(fc)
                    nc.vector.tensor_copy(
                        dtile[csl(mt * 128, (mt + 1) * 128)], pt[:])

        def ln1_dst(fc):
            return (xnT8[fc // 2],
                    lambda a, b, i=fc % 2: np.s_[:, i, a:b])

        def qk_block(nb):
            for w_sb, bias_col, dstT, sc in (
                    (wq_sb, bqc, qT, SQK / (SX * SWQ)),
                    (wk_sb, bkc, kT, SQK / (SX * SWK))):
                for mc in range(FC):
                    ps = psH.tile([128, 512], f32, tag="psH", name="psqk")
                    for kcp in range(KCP):
                        nc.tensor.matmul(
                            ps[:],
                            w_sb[kcp][:, :, mc * 128:(mc + 1) * 128],
                            xnT8[kcp][:, :, nb * 512:(nb + 1) * 512],
                            start=(kcp == 0), stop=(kcp == KCP - 1),
                            perf_mode=PM.DoubleRow)
                    nc.vector.tensor_scalar(
                        out=dstT[mc][:, nb * 512:(nb + 1) * 512], in0=ps[:],
                        scalar1=sc, scalar2=bias_col[:, mc:mc + 1],
                        op0=ALU.mult, op1=ALU.add)

        def v_block(mt):
            for nb in range(2):          # 6 heads (384 cols) per block
                ps = psV.tile([128, 384], f32, tag="psV", name="psv")
                for kcp in range(KCP):
                    nc.tensor.matmul(
                        ps[:],
                        xnT8[kcp][:, :, mt * 128:(mt + 1) * 128],
                        wv_sb[kcp][:, :, nb * 384:(nb + 1) * 384],
                        start=(kcp == 0), stop=(kcp == KCP - 1),
                        perf_mode=PM.DoubleRow)
                nc.vector.scalar_tensor_tensor(
                    vaug8[:, mt, nb * 6:(nb + 1) * 6, 0:D],
                    ps[:].rearrange("p (h e) -> p h e", h=6),
                    SV / (SX * SWV),
                    bv32_b[:, nb * 384:(nb + 1) * 384].rearrange(
                        "p (h e) -> p h e", h=6),
                    ALU.mult, ALU.add)
            nc.vector.memset(vaug8[:, mt, :, D:D + 1], 1.0)

        ln_tiles(xt[0:4], 0, ln1_dst, SX, tmpH, psH, "psHT")
        qk_block(0)
        ln_tiles(xt[4:8], 4, ln1_dst, SX, tmpH, psH, "psHT")
        qk_block(1)
        qk_stack.close()   # frees psH (PSUM) before the fill opens psS
        h_stack.close()    # frees xf32, wqk, tmpH (left SBUF)

        # proj + MLP weights (DMAs run during attention)
        w_stack = ExitStack()
        wo_pool = w_stack.enter_context(tc.tile_pool(name="wo8", bufs=KCP))
        w1_pool = w_stack.enter_context(tc.tile_pool(name="w1", bufs=FC))
        w2_pool = w_stack.enter_context(tc.tile_pool(name="w2", bufs=FH))
        wo_sb = [wo_pool.tile([128, 2, C], f8, tag="wo8", name="wo8")
                 for _ in range(KCP)]
        for kcp in range(KCP):
            nc.sync.dma_start(
                wo_sb[kcp][:],
                wo_d[kcp * 256:(kcp + 1) * 256, :].rearrange(
                    "(two p) m -> p two m", two=2))
        w1_sb = [w1_pool.tile([128, HID], bf16, tag="w1", name="w1")
                 for _ in range(FC)]
        for kc in range(FC):
            nc.sync.dma_start(w1_sb[kc][:], w1_d[kc * 128:(kc + 1) * 128, :])
        w2_sb = [w2_pool.tile([128, C], bf16, tag="w2", name="w2")
                 for _ in range(FH)]
        for kc in range(FH):
            nc.sync.dma_start(w2_sb[kc][:], w2_d[kc * 128:(kc + 1) * 128, :])

        # ================= main loop pools =================
        # PSUM bank budget (8): psS 2x2 + psV 2 (fill) -> psS 4 + psA 2 + psX 2
        m_stack = ExitStack()
        e_pool = m_stack.enter_context(tc.tile_pool(name="expS8", bufs=13))
        oa_pool = m_stack.enter_context(tc.tile_pool(name="oa", bufs=6))
        on_pool = m_stack.enter_context(tc.tile_pool(name="oTn8", bufs=2 * KCP))
        rr_pool = m_stack.enter_context(tc.tile_pool(name="rrec", bufs=2))
        xn2_pool = m_stack.enter_context(tc.tile_pool(name="xn2T", bufs=2 * FC))
        h8_pool = m_stack.enter_context(tc.tile_pool(name="hT", bufs=FH))
        ot_pool = m_stack.enter_context(tc.tile_pool(name="outs", bufs=2))
        tmpM = m_stack.enter_context(tc.tile_pool(name="tmpM", bufs=2))
        psS = m_stack.enter_context(
            tc.tile_pool(name="psS", bufs=2, space="PSUM"))

        def s_exp(h, q, expS):
            """S^T then exp for head h, query quarter q -> expS [128,NT,QT]."""
            mc, off = h // 2, (h % 2) * D
            for kth in range(2):
                ps = psS.tile([128, 4, QT], f32, tag="psS", name="psS")
                for k4 in range(4):
                    kt = 4 * kth + k4
                    nc.tensor.matmul(
                        ps[:, k4, :],
                        kT[mc][off:off + D, kt * 128:(kt + 1) * 128],
                        qT[mc][off:off + D, q * QT:(q + 1) * QT],
                        start=True, stop=True)
                nc.scalar.activation(
                    expS[:, 4 * kth:4 * kth + 4, :], ps[:], AF.Exp,
                    bias=shift_t[:], scale=1.0 / (SQK * SQK))

        def av(h, expS, oa):
            po = psA.tile([D + 1, QT], f32, tag="psA", name="psO")
            for ktp in range(4):
                nc.tensor.matmul(
                    po[:],
                    vaug8[:, 2 * ktp:2 * ktp + 2, h, 0:D + 1],
                    expS[:, 2 * ktp:2 * ktp + 2, :],
                    start=(ktp == 0), stop=(ktp == 3),
                    perf_mode=PM.DoubleRow)
            nc.vector.tensor_copy(oa[:], po[:])

        def pair_recip(oa_even, oa_odd):
            rs2_bf = rr_pool.tile([2, QT], bf16, tag="rs2b", name="rs2b")
            nc.sync.dma_start(rs2_bf[0:1, :], oa_even[D:D + 1, :])
            nc.sync.dma_start(rs2_bf[1:2, :], oa_odd[D:D + 1, :])
            rs2 = rr_pool.tile([2, QT], f32, tag="rf32", name="rs2")
            nc.vector.tensor_copy(rs2[:], rs2_bf[:])
            rr2 = rr_pool.tile([2, QT], f32, tag="rf32", name="rr2")
            nc.vector.reciprocal_approx_fast(rr2[:], rs2[:])
            rr2_bf = rr_pool.tile([2, QT], bf16, tag="rr2b", name="rr2b")
            nc.vector.tensor_copy(rr2_bf[:], rr2[:])
            return rr2_bf

        def pair_norm(j, oTn_q, oa_even, oa_odd, rr2_bf):
            pb = psA.tile([128, QT], f32, tag="psA", name="psR")
            nc.tensor.matmul(pb[:], ind2[:], rr2_bf[:], start=True, stop=True)
            kcp, i = j // 2, j % 2
            nc.vector.tensor_mul(oTn_q[kcp][0:D, i, :], oa_even[0:D, :],
                                 pb[0:D, :])
            nc.vector.tensor_mul(oTn_q[kcp][D:2 * D, i, :], oa_odd[0:D, :],
                                 pb[D:2 * D, :])

        expS_t = {}

        def emit_s_exp(h, q):
            expS_t[(h, q)] = e = e_pool.tile([128, NT, QT], f8,
                                             tag="expS", name="expS")
            s_exp(h, q, e)

        v_stack = ExitStack()
        psV = v_stack.enter_context(
            tc.tile_pool(name="psV", bufs=2, space="PSUM"))

        # fill: S/exp for quarter 0 riding alongside the v GEMMs; the
        # scheduler slots v matmuls into the exp-paced psS stall windows.
        for h in range(HEADS):
            emit_s_exp(h, 0)
            if h < NT:
                v_block(h)
        v_stack.close()   # frees psV banks for psA/psX
        hv_stack.close()  # frees xnT8 + wv (right SBUF)

        psAX_stack = ExitStack()
        psA = psAX_stack.enter_context(
            tc.tile_pool(name="psA", bufs=2, space="PSUM"))
        psX = psAX_stack.enter_context(
            tc.tile_pool(name="psX", bufs=2, space="PSUM"))

        xn2T_q = {}

        def attention(q):
            for mt01 in range(2):
                mt = 2 * q + mt01
                nc.gpsimd.tensor_add(xt[mt][:], xt[mt][:], bo_b[:])
            oTn_q = [on_pool.tile([128, 2, QT], f8, tag="oTn8", name="oTn8")
                     for _ in range(KCP)]
            oa_t = {}
            rr_prev = None
            for j in range(6):
                for i in range(2):
                    h = 2 * j + i
                    oa_t[h] = oa_pool.tile([D + 1, QT], bf16, tag="oa",
                                           name="oa")
                    av(h, expS_t.pop((h, q)), oa_t[h])
                if j >= 1:
                    pair_norm(j - 1, oTn_q, oa_t.pop(2 * j - 2),
                              oa_t.pop(2 * j - 1), rr_prev)
                rr_prev = pair_recip(oa_t[2 * j], oa_t[2 * j + 1])
            pair_norm(5, oTn_q, oa_t.pop(10), oa_t.pop(11), rr_prev)

            # proj(q)
            for mt01 in range(2):
                mt = 2 * q + mt01
                for nb in range(2):
                    ps = psX.tile([128, 384], f32, tag="psX", name="psP")
                    for kcp in range(KCP):
                        nc.tensor.matmul(
                            ps[:],
                            oTn_q[kcp][:, :, mt01 * 128:(mt01 + 1) * 128],
                            wo_sb[kcp][:, :, nb * 384:(nb + 1) * 384],
                            start=(kcp == 0), stop=(kcp == KCP - 1),
                            perf_mode=PM.DoubleRow)
                    nc.vector.scalar_tensor_tensor(
                        xt[mt][:, nb * 384:(nb + 1) * 384], ps[:],
                        1.0 / (SV * SWO),
                        xt[mt][:, nb * 384:(nb + 1) * 384],
                        ALU.mult, ALU.add)

            # LN2(q) -> xn2T (consumed by fc1 next iteration)
            xn2T_q[q] = xn2T = [
                xn2_pool.tile([128, QT], bf16, tag="xn2T", name="xn2T")
                for _ in range(FC)]

            def ln2_dst(fc):
                return (xn2T[fc],
                        lambda a, b, q=q: np.s_[:, a - 2 * q * 128:
                                                b - 2 * q * 128])

            ln_tiles(xt[2 * q:2 * q + 2], 2 * q, ln2_dst, 1.0, tmpM, psX,
                     "psX")

        def mlp(q):
            xn2T = xn2T_q.pop(q)
            hT = [h8_pool.tile([128, QT], bf16, tag="hT", name="hT")
                  for _ in range(FH)]
            for mc in range(FH):
                ps = psX.tile([128, QT], f32, tag="psX", name="psF")
                for kc in range(FC):
                    nc.tensor.matmul(
                        ps[:],
                        w1_sb[kc][:, mc * 128:(mc + 1) * 128],
                        xn2T[kc][:],
                        start=(kc == 0), stop=(kc == FC - 1))
                nc.scalar.activation(
                    hT[mc][:], ps[:], AF.Gelu, bias=b1c[:, mc:mc + 1])
            for mt01 in range(2):
                mt = 2 * q + mt01
                for nb in range(2):
                    ps = psX.tile([128, 384], f32, tag="psX", name="psF2")
                    for kc in range(FH):
                        nc.tensor.matmul(
                            ps[:],
                            hT[kc][:, mt01 * 128:(mt01 + 1) * 128],
                            w2_sb[kc][:, nb * 384:(nb + 1) * 384],
                            start=(kc == 0), stop=(kc == FH - 1))
                    ot = ot_pool.tile([128, 384], f32, tag="outs",
                                      name="outs")
                    nc.vector.tensor_add(
                        ot[:], ps[:], xt[mt][:, nb * 384:(nb + 1) * 384])
                    nc.sync.dma_start(
                        out_d[mt * 128:(mt + 1) * 128,
                              nb * 384:(nb + 1) * 384], ot[:])

        # steady state: attention(i) + S/exp(i+1) + MLP(i-1) per iteration.
        # ScalarE sees (by priority): sqrts(i) < exps(i+1) < gelus(i-1), and
        # stays saturated through the exp stream, so the priority heap keeps
        # each activation-table batch contiguous.
        for i in range(NQ + 1):
            if i < NQ:
                attention(i)
                if i + 1 < NQ:
                    for h in range(HEADS):
                        emit_s_exp(h, i + 1)
            if i >= 1:
                mlp(i - 1)

        psAX_stack.close()
        m_stack.close()
        w_stack.close()
        a_stack.close()

    nc.compile()
    return nc


def _prep_inputs(inputs):
    """Host-side algebraic folds + fp8/bf16 casts. Returns per-core in_maps."""
    f = {k: np.asarray(v, np.float32) for k, v in inputs.items()}
    bf = ml_dtypes.bfloat16
    f8 = ml_dtypes.float8_e4m3
    d = 1.0 / np.sqrt(C // HEADS)

    wq8 = (f["ln1_g"][:, None] * f["Wq"] * d * SWQ).astype(f8)
    bq = ((f["bq"] + f["ln1_b"] @ f["Wq"]) * d * SQK).astype(np.float32)
    wk8 = (f["ln1_g"][:, None] * f["Wk"] * SWK).astype(f8)
    bk = ((f["bk"] + f["ln1_b"] @ f["Wk"]) * SQK).astype(np.float32)
    wv8 = (f["ln1_g"][:, None] * f["Wv"] * SWV).astype(f8)
    bv32 = ((f["bv"] + f["ln1_b"] @ f["Wv"]) * SV).astype(bf)
    wo8 = (f["Wo"] * SWO).astype(f8)
    w1 = (f["ln2_g"][:, None] * f["W1"]).astype(bf)
    b1 = (f["b1"] + f["ln2_b"] @ f["W1"]).astype(np.float32)
    shared = {
        "wq": wq8, "bq": bq, "wk": wk8, "bk": bk, "wv": wv8, "bv32": bv32,
        "wo": wo8, "bo": f["bo"].astype(bf),
        "w1": w1, "b1": b1,
        "w2": f["W2"].astype(bf), "b2": f["b2"].astype(bf),
    }
    ind2 = np.zeros((2, 128), bf)
    ind2[0, 0:64] = 1.0
    ind2[1, 64:128] = 1.0
    shared["ind2"] = ind2
    x = f["x"]
    return [dict(shared, x=np.ascontiguousarray(x[i])) for i in range(N_CORES)]


def kernel(**inputs):
    from concourse.bass_utils import run_bass_kernel_spmd
    if "nc" not in _CACHE:
        _CACHE["nc"] = _build()
    nc = _CACHE["nc"]
    in_maps = _prep_inputs(inputs)
    res = run_bass_kernel_spmd(nc, in_maps, core_ids=list(range(N_CORES)))
    out = np.stack([np.asarray(res.results[i]["out"], np.float32)
                    for i in range(N_CORES)])
    return out
